# revision 8
# baseline (speedup 1.0000x reference)
"""AdaLoRA MLP with base — 8-core Trainium2 Bass kernel.

Strategy (tensor-parallel W2 + d-sharded LoRA factors):
  - The dominant cost is reading W2 [1024, 32768] f32 (134 MB). Each core owns
    a 1/8 column slice (respecting the 4 chunk boundaries la1|lb1|la2|lb2),
    stored bf16, cg-major, and computes w_part = h @ W2_i for ALL 128 samples.
  - Per-sample LoRA factors then live d-sharded: rank i holds la*/lb*[b, d, r]
    for d in Di = [128*i, 128*(i+1)). All contractions over d become rank-local
    partial sums + tiny collectives:
      t2 = sum_d la2*x   -> AllReduce [128,8]
      y  = x@bd[:,Di] + sum_r lb2*t2 (gelu)        (column-local, no comm)
      t1 = sum_l lb1*y   -> AllReduce [128,8]
      term1 = y_i @ base_up[:,Di].T (partial sum)  -> ReduceScatter [128,1024]
      term2 = sum_r la1*t1 (k-local)               -> AllToAll [128,128]
  - Output: rank i returns out[16*i:16*(i+1), :]; host concatenates.

h-path: LayerNorm on-chip (g/beta folded into W1/b1 on host), h computed
directly transposed (hT[a,b]) in bf16 so it feeds the W2 matmul as stationary.
"""

import sys

sys.path.insert(0, "/opt/trn_rl_repo")

import numpy as np
import ml_dtypes

import concourse.bass as bass
import concourse.tile as tile
from concourse import mybir, bass_utils
from concourse.bass import ts
from concourse.masks import make_identity
from concourse.vector_clock import ScopedClock

B, D, ADA, R = 128, 1024, 1024, 8
NCORES = 8
BP = B // NCORES      # samples per core = 16
DP = D // NCORES      # d-slice per core = 128
FP32 = mybir.dt.float32
BF16 = mybir.dt.bfloat16
AF = mybir.ActivationFunctionType
ALU = mybir.AluOpType

# walrus in this environment rejects InstDrain carrying sem-eq waits or >1
# wait command; route barrier waits through EventSemaphores and split the
# tail drain's waits into individual wait_ge instructions.
_PATCHED = False


def _apply_patches():
    global _PATCHED
    if _PATCHED:
        return
    _PATCHED = True

    def multi_engine_barrier(self, engines):
        for e in engines:
            self.engines[e].drain()
        tag = f"mbar{self.next_id()}"
        for inst in self._sem_only_all_engine_barrier_insts(tag):
            self.engines[inst.engine].add_instruction(inst)

    bass.Bass.multi_engine_barrier = multi_engine_barrier

    def _drain_and_barrier(self, tick_clock, wait_clock):
        nc = self.nc
        drain_inst = nc.sync.drain()
        wait_clock.add_sem_waits(
            drain_inst.ins, ScopedClock({None: tick_clock.global_clock})
        )
        si = drain_inst.ins.sync_info
        waits = list(si.on_wait or []) if si else []
        if len(waits) > 1:
            drain_inst.ins.sync_info = mybir.SyncInfo(
                on_wait=[waits[0]], on_update=list(si.on_update or [])
            )
            handle_map = {h.num: h for h in self.sems.allocated().values()}
            for w in waits[1:]:
                h = handle_map.get(w.id)
                assert h is not None and w.wait_mode == "sem-ge-imm"
                nc.sync.wait_ge(h, w.wait_value)
        nc.all_engine_barrier()
        popped = nc._tile_sem_poison_stack.pop()
        assert popped is self._sem_poison
        nc.clear_and_free_semaphores(list(self.sems.allocated().values()))
        nc.all_engine_barrier()

    tile.TileContext._drain_and_barrier = _drain_and_barrier


def _split_excess_waits(nc, max_waits=1):
    """walrus here encodes at most ~1 sync-wait command per instruction;
    hoist excess waits onto NoOp carriers placed just before (same engine)."""
    for fn in nc.m.functions:
        for bb in fn.blocks:
            new = []
            changed = False
            for ins in bb.instructions:
                si = ins.sync_info
                waits = list(si.on_wait or []) if si else []
                if len(waits) > max_waits:
                    changed = True
                    for w in waits[:-max_waits]:
                        nop = mybir.InstNoOp(name=f"I-wsplit-{nc.next_id()}")
                        nop.engine = ins.engine
                        nop.sync_info = mybir.SyncInfo(on_wait=[w], on_update=[])
                        nc.register_instruction(nop, overwrite=True)
                        new.append(nop)
                    ins.sync_info = mybir.SyncInfo(
                        on_wait=waits[-max_waits:], on_update=list(si.on_update or [])
                    )
                new.append(ins)
            if changed:
                bb.instructions = new


DEBUG = False


def build_graph():
    _apply_patches()
    nc = bass.Bass("TRN2", target_bir_lowering=False, debug=False, num_devices=NCORES)
    RG = [list(range(NCORES))]

    ada = nc.dram_tensor("ada", [B, ADA], FP32, kind="ExternalInput").ap()
    w1 = nc.dram_tensor("w1", [ADA, ADA], BF16, kind="ExternalInput").ap()
    b1c = nc.dram_tensor("b1c", [128, 8], FP32, kind="ExternalInput").ap()
    w2 = nc.dram_tensor("w2", [8, ADA, 512], BF16, kind="ExternalInput").ap()
    b2r = nc.dram_tensor("b2r", [1, 4096], BF16, kind="ExternalInput").ap()
    xT = nc.dram_tensor("xT", [D, B], FP32, kind="ExternalInput").ap()
    xcol = nc.dram_tensor("xcol", [B, DP], FP32, kind="ExternalInput").ap()
    bd = nc.dram_tensor("bd", [D, DP], FP32, kind="ExternalInput").ap()
    buT = nc.dram_tensor("buT", [DP, D], FP32, kind="ExternalInput").ap()
    xres = nc.dram_tensor("xres", [BP, D], FP32, kind="ExternalInput").ap()
    outp = nc.dram_tensor("out", [BP, D], FP32, kind="ExternalOutput").ap()
    dbg = {}
    if DEBUG:
        for nm, shp in [("dbg_c", [B, ADA]), ("dbg_hT", [128, ADA]),
                        ("dbg_wp", [B, 4096]), ("dbg_t2p", [B, R]),
                        ("dbg_t2", [B, R]), ("dbg_y", [B, DP]),
                        ("dbg_t1", [B, R]), ("dbg_rs", [B, D]),
                        ("dbg_term2", [B, DP]), ("dbg_rsout", [BP, D]),
                        ("dbg_a2a", [BP, D])]:
            dbg[nm] = nc.dram_tensor(nm, shp, FP32, kind="ExternalOutput").ap()

    with tile.TileContext(nc) as tc:
        with (
            tc.tile_pool(name="singles", bufs=1) as S,
            tc.tile_pool(name="small", bufs=2) as SM,
            tc.tile_pool(name="w1pool", bufs=1) as W1P,
            tc.tile_pool(name="w2pool", bufs=6) as W2P,
            tc.tile_pool(name="ph", bufs=2, space="PSUM") as PH,
            tc.tile_pool(name="pw", bufs=3, space="PSUM") as PW,
            tc.tile_pool(name="pmisc", bufs=2, space="PSUM") as PM,
            tc.tile_pool(name="dram", bufs=1, space="DRAM") as DR,
        ):
            # ---------------- Phase A: LayerNorm + transpose c ----------
            ada_sb = S.tile([B, ADA], FP32)
            nc.sync.dma_start(out=ada_sb[:], in_=ada)

            eps = S.tile([B, 1], FP32)
            nc.vector.memset(eps[:], 1e-5)
            stats = SM.tile([B, 2, nc.vector.BN_STATS_DIM], FP32)
            for sg in range(2):
                nc.vector.bn_stats(
                    out=stats[:, sg, :], in_=ada_sb[:, ts(sg, 512)]
                )
            mv = SM.tile([B, nc.vector.BN_AGGR_DIM], FP32)
            nc.vector.bn_aggr(out=mv[:], in_=stats[:])
            rstd = SM.tile([B, 1], FP32)
            nc.scalar.activation(
                out=rstd[:], in_=mv[:, 1:2], func=AF.Sqrt, bias=eps[:], scale=1.0
            )
            nc.vector.reciprocal(out=rstd[:], in_=rstd[:])
            c_bf = S.tile([B, ADA], BF16)
            nc.vector.tensor_scalar(
                out=c_bf[:],
                in0=ada_sb[:],
                scalar1=mv[:, 0:1],
                scalar2=rstd[:],
                op0=ALU.subtract,
                op1=ALU.mult,
            )

            if DEBUG:
                cdump = SM.tile([B, ADA], FP32)
                nc.vector.tensor_copy(cdump[:], c_bf[:])
                nc.sync.dma_start(out=dbg["dbg_c"], in_=cdump[:])
            ident_bf = S.tile([128, 128], BF16)
            make_identity(nc, ident_bf[:])
            cT = S.tile([128, ADA], BF16)  # 8 tiles of c.T
            for e in range(8):
                p = PM.tile([128, 512], BF16, tag="ps")
                nc.tensor.transpose(p[:, 0:128], c_bf[:, ts(e, 128)], ident_bf[:])
                nc.vector.tensor_copy(cT[:, ts(e, 128)], p[:, 0:128])

            # ---------------- Phase B: hT = gelu(W1'.T @ c.T + b1') -----
            b1c_sb = S.tile([128, 8], FP32)
            nc.sync.dma_start(out=b1c_sb[:], in_=b1c)
            # one single-bank PSUM accumulation group per a-tile: start=True
            # clears the whole bank, so groups must not interleave in a bank
            w1rows = []
            for e in range(8):
                w1row = W1P.tile([128, ADA], BF16, tag=f"w1r{e}")
                nc.sync.dma_start(out=w1row[:], in_=w1[ts(e, 128), :])
                w1rows.append(w1row)
            hT = S.tile([128, ADA], BF16)
            for a in range(8):
                pha = PH.tile([128, 128], FP32, tag="pha")
                for e in range(8):
                    nc.tensor.matmul(
                        pha[:],
                        w1rows[e][:, ts(a, 128)],
                        cT[:, ts(e, 128)],
                        start=(e == 0),
                        stop=(e == 7),
                    )
                nc.scalar.activation(
                    out=hT[:, ts(a, 128)],
                    in_=pha[:],
                    func=AF.Gelu,
                    bias=b1c_sb[:, a : a + 1],
                    scale=1.0,
                )

            if DEBUG:
                hdump = SM.tile([128, ADA], FP32)
                nc.vector.tensor_copy(hdump[:], hT[:])
                nc.sync.dma_start(out=dbg["dbg_hT"], in_=hdump[:])
            # ---------------- Phase C: w_part = h @ W2_i + b2_i ---------
            # slab cols: [la2 | lb2 | lb1 | la1], within chunk col = r*128+dl
            ones_bf = S.tile([1, 128], BF16)
            nc.vector.memset(ones_bf[:], 1.0)
            b2_sb = S.tile([1, 4096], BF16)
            nc.sync.dma_start(out=b2_sb[:], in_=b2r)
            wp = S.tile([B, 4096], FP32)
            for cg in range(8):
                pwt = PW.tile([B, 512], FP32, tag="pwt")
                for a in range(8):
                    slab = W2P.tile([128, 512], BF16)
                    nc.sync.dma_start(out=slab[:], in_=w2[cg, ts(a, 128), :])
                    nc.tensor.matmul(
                        pwt[:],
                        hT[:, ts(a, 128)],
                        slab[:],
                        start=(a == 0),
                        stop=False,
                    )
                nc.tensor.matmul(
                    pwt[:], ones_bf[:], b2_sb[:, ts(cg, 512)], start=False, stop=True
                )
                nc.vector.tensor_copy(wp[:, ts(cg, 512)], pwt[:])

            if DEBUG:
                nc.sync.dma_start(out=dbg["dbg_wp"], in_=wp[:])
            # ---------------- t2 = sum_d la2*x -> AllReduce -------------
            xcol_sb = S.tile([B, DP], FP32)
            nc.sync.dma_start(out=xcol_sb[:], in_=xcol)
            prod = S.tile([B, 1024], FP32)
            for r in range(R):
                nc.vector.tensor_mul(
                    prod[:, ts(r, 128)], wp[:, ts(r, 128)], xcol_sb[:]
                )
            t2p = SM.tile([B, R], FP32)
            nc.vector.reduce_sum(
                out=t2p[:],
                in_=prod[:].rearrange("p (r d) -> p r d", d=128),
                axis=mybir.AxisListType.X,
            )
            t2in = DR.tile([B, R], FP32)
            t2out = DR.tile([B, R], FP32)
            nc.sync.dma_start(out=t2in[:], in_=t2p[:])
            nc.gpsimd.collective_compute(
                "AllReduce", ALU.add, replica_groups=RG,
                ins=[t2in.opt()], outs=[t2out.opt()],
            )
            t2_sb = SM.tile([B, R], FP32)
            nc.sync.dma_start(out=t2_sb[:], in_=t2out[:])
            if DEBUG:
                nc.sync.dma_start(out=dbg["dbg_t2p"], in_=t2p[:])
                nc.sync.dma_start(out=dbg["dbg_t2"], in_=t2_sb[:])

            # ---------------- y = gelu(x@bd + sum_r lb2*t2) -------------
            xT_sb = S.tile([128, D], FP32)
            nc.sync.dma_start(
                out=xT_sb[:].rearrange("p (t b) -> p t b", t=8),
                in_=xT.rearrange("(t p) b -> p t b", p=128),
            )
            bd_sb = S.tile([128, D], FP32)
            nc.sync.dma_start(
                out=bd_sb[:].rearrange("p (t l) -> p t l", t=8),
                in_=bd.rearrange("(t p) l -> p t l", p=128),
            )
            py0 = PM.tile([B, 512], FP32, tag="ps")
            for dt_ in range(8):
                nc.tensor.matmul(
                    py0[:, 0:128],
                    xT_sb[:, ts(dt_, 128)],
                    bd_sb[:, ts(dt_, 128)],
                    start=(dt_ == 0),
                    stop=(dt_ == 7),
                )
            y_acc = SM.tile([B, DP], FP32)
            nc.vector.tensor_copy(y_acc[:], py0[:, 0:128])
            tmp = SM.tile([B, DP], FP32)
            for r in range(R):
                nc.vector.tensor_scalar_mul(
                    out=tmp[:], in0=wp[:, 1024 + r * 128 : 1024 + (r + 1) * 128],
                    scalar1=t2_sb[:, r : r + 1],
                )
                nc.vector.tensor_add(y_acc[:], y_acc[:], tmp[:])
            y_sb = S.tile([B, DP], FP32)
            nc.scalar.activation(out=y_sb[:], in_=y_acc[:], func=AF.Gelu)

            if DEBUG:
                nc.sync.dma_start(out=dbg["dbg_y"], in_=y_sb[:])
            # ---------------- t1 = sum_l lb1*y -> AllReduce -------------
            prod2 = S.tile([B, 1024], FP32)
            for r in range(R):
                nc.vector.tensor_mul(
                    prod2[:, ts(r, 128)],
                    wp[:, 2048 + r * 128 : 2048 + (r + 1) * 128],
                    y_sb[:],
                )
            t1p = SM.tile([B, R], FP32)
            nc.vector.reduce_sum(
                out=t1p[:],
                in_=prod2[:].rearrange("p (r d) -> p r d", d=128),
                axis=mybir.AxisListType.X,
            )
            t1in = DR.tile([B, R], FP32)
            t1out = DR.tile([B, R], FP32)
            nc.sync.dma_start(out=t1in[:], in_=t1p[:])
            nc.gpsimd.collective_compute(
                "AllReduce", ALU.add, replica_groups=RG,
                ins=[t1in.opt()], outs=[t1out.opt()],
            )
            t1_sb = SM.tile([B, R], FP32)
            nc.sync.dma_start(out=t1_sb[:], in_=t1out[:])

            if DEBUG:
                nc.sync.dma_start(out=dbg["dbg_t1"], in_=t1_sb[:])
            # ---------------- term1 = y_i @ buT -> ReduceScatter --------
            ident_f = S.tile([128, 128], FP32)
            make_identity(nc, ident_f[:])
            pyT = PM.tile([DP, 512], FP32, tag="ps")
            nc.tensor.transpose(pyT[:, 0:128], y_sb[:], ident_f[:])
            yT_sb = SM.tile([DP, B], FP32)
            nc.vector.tensor_copy(yT_sb[:], pyT[:, 0:128])
            buT_sb = S.tile([DP, D], FP32)
            nc.sync.dma_start(out=buT_sb[:], in_=buT)
            rs_sb = S.tile([B, D], FP32)
            for n in range(2):
                pterm = PM.tile([B, 512], FP32, tag="ps")
                nc.tensor.matmul(
                    pterm[:], yT_sb[:], buT_sb[:, ts(n, 512)], start=True, stop=True
                )
                nc.vector.tensor_copy(rs_sb[:, ts(n, 512)], pterm[:])
            if DEBUG:
                nc.sync.dma_start(out=dbg["dbg_rs"], in_=rs_sb[:])
            rsin = DR.tile([B, D], FP32)
            rsout = DR.tile([BP, D], FP32)
            nc.sync.dma_start(out=rsin[:], in_=rs_sb[:])
            nc.gpsimd.collective_compute(
                "ReduceScatter", ALU.add, replica_groups=RG,
                ins=[rsin.opt()], outs=[rsout.opt()],
            )

            # ---------------- term2 = sum_r la1*t1 -> AllToAll ----------
            term2 = SM.tile([B, DP], FP32)
            tmp2 = SM.tile([B, DP], FP32)
            for r in range(R):
                sl = wp[:, 3072 + r * 128 : 3072 + (r + 1) * 128]
                if r == 0:
                    nc.vector.tensor_scalar_mul(
                        out=term2[:], in0=sl, scalar1=t1_sb[:, r : r + 1]
                    )
                else:
                    nc.vector.tensor_scalar_mul(
                        out=tmp2[:], in0=sl, scalar1=t1_sb[:, r : r + 1]
                    )
                    nc.vector.tensor_add(term2[:], term2[:], tmp2[:])
            if DEBUG:
                nc.sync.dma_start(out=dbg["dbg_term2"], in_=term2[:])
            a2in = DR.tile([B, DP], FP32)
            a2out = DR.tile([B, DP], FP32)
            nc.sync.dma_start(out=a2in[:], in_=term2[:])
            nc.gpsimd.collective_compute(
                "AllToAll", ALU.bypass, replica_groups=RG,
                ins=[a2in.opt()], outs=[a2out.opt()],
            )

            # ---------------- final: out = x + rsout + term2(asm) -------
            fin = SM.tile([BP, D], FP32)
            # a2out rows = [src_rank j][16 samples]; cols = that rank's k-slice
            nc.sync.dma_start(
                out=fin[:].rearrange("b (j k) -> b j k", j=8),
                in_=a2out[:].rearrange("(j b) k -> b j k", j=8),
            )
            rs_res = SM.tile([BP, D], FP32)
            nc.sync.dma_start(out=rs_res[:], in_=rsout[:])
            xres_sb = SM.tile([BP, D], FP32)
            nc.sync.dma_start(out=xres_sb[:], in_=xres)
            if DEBUG:
                nc.sync.dma_start(out=dbg["dbg_rsout"], in_=rs_res[:])
                findump = SM.tile([BP, D], FP32)
                nc.vector.tensor_copy(findump[:], fin[:])
                nc.sync.dma_start(out=dbg["dbg_a2a"], in_=findump[:])
            nc.vector.tensor_add(fin[:], fin[:], rs_res[:])
            nc.vector.tensor_add(fin[:], fin[:], xres_sb[:])
            nc.sync.dma_start(out=outp, in_=fin[:])

    _split_excess_waits(nc)
    return nc


_NC = None


def _get_nc():
    global _NC
    if _NC is None:
        _NC = build_graph()
    return _NC


def make_in_maps(x, ada_emb, base_up, base_down, ln_g, ln_b, W1, b1, W2, b2):
    f32 = np.float32
    bf16 = ml_dtypes.bfloat16
    x = np.asarray(x, f32)
    ada_emb = np.ascontiguousarray(np.asarray(ada_emb, f32))
    W1 = np.asarray(W1, f32)
    W2 = np.asarray(W2, f32)
    # fold LN affine into W1/b1
    W1p = (np.asarray(ln_g, f32)[:, None] * W1).astype(bf16)
    b1p = (np.asarray(b1, f32) + np.asarray(ln_b, f32) @ W1).astype(f32)
    b1c = np.ascontiguousarray(b1p.reshape(8, 128).T)
    xT = np.ascontiguousarray(x.T)
    W2r = W2.reshape(ADA, 4, D, R)
    b2r_full = np.asarray(b2, f32).reshape(4, D, R)
    chunk_order = [2, 3, 1, 0]  # la2, lb2, lb1, la1

    in_maps = []
    for i in range(NCORES):
        dl = slice(i * DP, (i + 1) * DP)
        sl = W2r[:, chunk_order, dl, :]            # [a, 4, 128, 8]
        sl = sl.transpose(1, 3, 2, 0)              # [4, r, dl, a]
        # cols (chunk, r, dl): slab[a, chunk*1024 + r*128 + dl]
        slab = np.ascontiguousarray(sl.reshape(4096, ADA).T)   # [a, 4096]
        w2i = np.ascontiguousarray(
            slab.reshape(ADA, 8, 512).transpose(1, 0, 2)
        ).astype(bf16)                              # [8, a, 512]
        b2sl = b2r_full[chunk_order, dl, :]         # [4, 128, 8]
        b2i = np.ascontiguousarray(
            b2sl.transpose(0, 2, 1).reshape(1, 4096)
        ).astype(bf16)
        in_maps.append(
            {
                "ada": ada_emb,
                "w1": W1p,
                "b1c": b1c,
                "w2": w2i,
                "b2r": b2i,
                "xT": xT,
                "xcol": np.ascontiguousarray(x[:, dl]),
                "bd": np.ascontiguousarray(np.asarray(base_down, f32)[:, dl]),
                "buT": np.ascontiguousarray(np.asarray(base_up, f32)[:, dl].T),
                "xres": np.ascontiguousarray(x[i * BP : (i + 1) * BP, :]),
            }
        )
    return in_maps


def run(trace=False, tmpdir=None, **inputs):
    nc = _get_nc()
    in_maps = make_in_maps(**inputs)
    res = bass_utils.run_bass_kernel_spmd(
        nc, in_maps, core_ids=list(range(NCORES)), trace=trace, tmpdir=tmpdir
    )
    out = np.concatenate([res.results[i]["out"] for i in range(NCORES)], axis=0)
    return out, res


def kernel(**inputs):
    out, _ = run(trace=False, **inputs)
    return out


# revision 11
# speedup vs baseline: 1.0737x; 1.0737x over previous
"""AdaLoRA MLP with base — 8-core Trainium2 Bass kernel.

Strategy (tensor-parallel W2 + d-sharded LoRA factors):
  - The dominant cost is reading W2 [1024, 32768] f32 (134 MB). Each core owns
    a 1/8 column slice (respecting the 4 chunk boundaries la1|lb1|la2|lb2),
    stored bf16, cg-major, and computes w_part = h @ W2_i for ALL 128 samples.
  - Per-sample LoRA factors then live d-sharded: rank i holds la*/lb*[b, d, r]
    for d in Di = [128*i, 128*(i+1)). All contractions over d become rank-local
    partial sums + tiny collectives:
      t2 = sum_d la2*x   -> AllReduce [128,8]
      y  = x@bd[:,Di] + sum_r lb2*t2 (gelu)        (column-local, no comm)
      t1 = sum_l lb1*y   -> AllReduce [128,8]
      term1 = y_i @ base_up[:,Di].T (partial sum)  -> ReduceScatter [128,1024]
      term2 = sum_r la1*t1 (k-local)               -> AllToAll [128,128]
  - Output: rank i returns out[16*i:16*(i+1), :]; host concatenates.

h-path: LayerNorm on-chip (g/beta folded into W1/b1 on host), h computed
directly transposed (hT[a,b]) in bf16 so it feeds the W2 matmul as stationary.
"""

import sys

sys.path.insert(0, "/opt/trn_rl_repo")

import numpy as np
import ml_dtypes

import concourse.bass as bass
import concourse.tile as tile
from concourse import mybir, bass_utils
from concourse.bass import ts
from concourse.masks import make_identity
from concourse.vector_clock import ScopedClock

B, D, ADA, R = 128, 1024, 1024, 8
NCORES = 8
BP = B // NCORES      # samples per core = 16
DP = D // NCORES      # d-slice per core = 128
FP32 = mybir.dt.float32
BF16 = mybir.dt.bfloat16
AF = mybir.ActivationFunctionType
ALU = mybir.AluOpType

# walrus in this environment rejects InstDrain carrying sem-eq waits or >1
# wait command; route barrier waits through EventSemaphores and split the
# tail drain's waits into individual wait_ge instructions.
_PATCHED = False


def _apply_patches():
    global _PATCHED
    if _PATCHED:
        return
    _PATCHED = True

    def multi_engine_barrier(self, engines):
        for e in engines:
            self.engines[e].drain()
        tag = f"mbar{self.next_id()}"
        for inst in self._sem_only_all_engine_barrier_insts(tag):
            self.engines[inst.engine].add_instruction(inst)

    bass.Bass.multi_engine_barrier = multi_engine_barrier

    def _drain_and_barrier(self, tick_clock, wait_clock):
        nc = self.nc
        drain_inst = nc.sync.drain()
        wait_clock.add_sem_waits(
            drain_inst.ins, ScopedClock({None: tick_clock.global_clock})
        )
        si = drain_inst.ins.sync_info
        waits = list(si.on_wait or []) if si else []
        if len(waits) > 1:
            drain_inst.ins.sync_info = mybir.SyncInfo(
                on_wait=[waits[0]], on_update=list(si.on_update or [])
            )
            handle_map = {h.num: h for h in self.sems.allocated().values()}
            for w in waits[1:]:
                h = handle_map.get(w.id)
                assert h is not None and w.wait_mode == "sem-ge-imm"
                nc.sync.wait_ge(h, w.wait_value)
        nc.all_engine_barrier()
        popped = nc._tile_sem_poison_stack.pop()
        assert popped is self._sem_poison
        nc.clear_and_free_semaphores(list(self.sems.allocated().values()))

    tile.TileContext._drain_and_barrier = _drain_and_barrier


def _split_excess_waits(nc, max_waits=1):
    """walrus here encodes at most ~1 sync-wait command per instruction;
    hoist excess waits onto NoOp carriers placed just before (same engine)."""
    for fn in nc.m.functions:
        for bb in fn.blocks:
            new = []
            changed = False
            for ins in bb.instructions:
                si = ins.sync_info
                waits = list(si.on_wait or []) if si else []
                if len(waits) > max_waits:
                    changed = True
                    for w in waits[:-max_waits]:
                        nop = mybir.InstNoOp(name=f"I-wsplit-{nc.next_id()}")
                        nop.engine = ins.engine
                        nop.sync_info = mybir.SyncInfo(on_wait=[w], on_update=[])
                        nc.register_instruction(nop, overwrite=True)
                        new.append(nop)
                    ins.sync_info = mybir.SyncInfo(
                        on_wait=waits[-max_waits:], on_update=list(si.on_update or [])
                    )
                new.append(ins)
            if changed:
                bb.instructions = new


DEBUG = False


def build_graph():
    _apply_patches()
    nc = bass.Bass("TRN2", target_bir_lowering=False, debug=False, num_devices=NCORES)
    RG = [list(range(NCORES))]

    ada = nc.dram_tensor("ada", [B, ADA], FP32, kind="ExternalInput").ap()
    w1 = nc.dram_tensor("w1", [ADA, ADA], BF16, kind="ExternalInput").ap()
    b1c = nc.dram_tensor("b1c", [128, 8], FP32, kind="ExternalInput").ap()
    w2 = nc.dram_tensor("w2", [2, ADA, 2048], BF16, kind="ExternalInput").ap()
    b2r = nc.dram_tensor("b2r", [1, 4096], BF16, kind="ExternalInput").ap()
    xT = nc.dram_tensor("xT", [D, B], FP32, kind="ExternalInput").ap()
    xcol = nc.dram_tensor("xcol", [B, DP], FP32, kind="ExternalInput").ap()
    bd = nc.dram_tensor("bd", [D, DP], FP32, kind="ExternalInput").ap()
    buT = nc.dram_tensor("buT", [DP, D], FP32, kind="ExternalInput").ap()
    xres = nc.dram_tensor("xres", [BP, D], FP32, kind="ExternalInput").ap()
    eh = nc.dram_tensor("eh", [DP, D], BF16, kind="ExternalInput").ap()
    outp = nc.dram_tensor("out", [BP, D], FP32, kind="ExternalOutput").ap()
    dbg = {}
    if DEBUG:
        for nm, shp in [("dbg_c", [B, ADA]), ("dbg_hT", [128, ADA]),
                        ("dbg_wp", [B, 4096]), ("dbg_t2p", [B, R]),
                        ("dbg_t2", [B, R]), ("dbg_y", [B, DP]),
                        ("dbg_t1", [B, R]), ("dbg_rs", [B, D]),
                        ("dbg_term2", [B, DP]), ("dbg_rsout", [BP, D]),
                        ("dbg_a2a", [BP, D])]:
            dbg[nm] = nc.dram_tensor(nm, shp, FP32, kind="ExternalOutput").ap()

    with tile.TileContext(nc) as tc:
        with (
            tc.tile_pool(name="singles", bufs=1) as S,
            tc.tile_pool(name="small", bufs=2) as SM,
            tc.tile_pool(name="w1pool", bufs=1) as W1P,
            tc.tile_pool(name="w2pool", bufs=6) as W2P,
            tc.tile_pool(name="ph", bufs=2, space="PSUM") as PH,
            tc.tile_pool(name="pw", bufs=1, space="PSUM") as PW,
            tc.tile_pool(name="pmisc", bufs=2, space="PSUM") as PM,
            tc.tile_pool(name="dram", bufs=1, space="DRAM") as DR,
        ):
            # warmup collective: absorbs the ncfw init barrier + first-use
            # cost concurrently with the DMA/compute front
            wuin = DR.tile([1, 8], FP32)
            wuout = DR.tile([1, 8], FP32)
            nc.gpsimd.collective_compute(
                "AllReduce", ALU.add, replica_groups=RG,
                ins=[wuin.opt()], outs=[wuout.opt()],
            )
            # ---------------- Phase A: LayerNorm + transpose c ----------
            ada_sb = S.tile([B, ADA], FP32)
            nc.sync.dma_start(out=ada_sb[:], in_=ada)

            eps = S.tile([B, 1], FP32)
            nc.vector.memset(eps[:], 1e-5)
            stats = SM.tile([B, 2, nc.vector.BN_STATS_DIM], FP32)
            for sg in range(2):
                nc.vector.bn_stats(
                    out=stats[:, sg, :], in_=ada_sb[:, ts(sg, 512)]
                )
            mv = SM.tile([B, nc.vector.BN_AGGR_DIM], FP32)
            nc.vector.bn_aggr(out=mv[:], in_=stats[:])
            rstd = SM.tile([B, 1], FP32)
            nc.scalar.activation(
                out=rstd[:], in_=mv[:, 1:2], func=AF.Sqrt, bias=eps[:], scale=1.0
            )
            nc.vector.reciprocal(out=rstd[:], in_=rstd[:])
            c_bf = S.tile([B, ADA], BF16)
            nc.vector.tensor_scalar(
                out=c_bf[:],
                in0=ada_sb[:],
                scalar1=mv[:, 0:1],
                scalar2=rstd[:],
                op0=ALU.subtract,
                op1=ALU.mult,
            )

            if DEBUG:
                cdump = SM.tile([B, ADA], FP32)
                nc.vector.tensor_copy(cdump[:], c_bf[:])
                nc.sync.dma_start(out=dbg["dbg_c"], in_=cdump[:])
            ident_bf = S.tile([128, 128], BF16)
            make_identity(nc, ident_bf[:])
            cT = S.tile([128, ADA], BF16)  # 8 tiles of c.T
            for e in range(8):
                p = PM.tile([128, 512], BF16, tag="ps")
                nc.tensor.transpose(p[:, 0:128], c_bf[:, ts(e, 128)], ident_bf[:])
                nc.vector.tensor_copy(cT[:, ts(e, 128)], p[:, 0:128])

            # ---------------- Phase B: hT = gelu(W1'.T @ c.T + b1') -----
            b1c_sb = S.tile([128, 8], FP32)
            nc.sync.dma_start(out=b1c_sb[:], in_=b1c)
            # one single-bank PSUM accumulation group per a-tile: start=True
            # clears the whole bank, so groups must not interleave in a bank
            w1rows = []
            for e in range(8):
                w1row = W1P.tile([128, ADA], BF16, tag=f"w1r{e}")
                nc.sync.dma_start(out=w1row[:], in_=w1[ts(e, 128), :])
                w1rows.append(w1row)
            hT = S.tile([128, ADA], BF16)
            for a in range(8):
                pha = PH.tile([128, 128], FP32, tag="pha")
                for e in range(8):
                    nc.tensor.matmul(
                        pha[:],
                        w1rows[e][:, ts(a, 128)],
                        cT[:, ts(e, 128)],
                        start=(e == 0),
                        stop=(e == 7),
                    )
                nc.scalar.activation(
                    out=hT[:, ts(a, 128)],
                    in_=pha[:],
                    func=AF.Gelu,
                    bias=b1c_sb[:, a : a + 1],
                    scale=1.0,
                )

            if DEBUG:
                hdump = SM.tile([128, ADA], FP32)
                nc.vector.tensor_copy(hdump[:], hT[:])
                nc.sync.dma_start(out=dbg["dbg_hT"], in_=hdump[:])
            # ---------------- Phase C: w_part = h @ W2_i + b2_i ---------
            # slab cols: [la2 | lb2 | lb1 | la1], within chunk col = r*128+dl
            ones_bf = S.tile([1, 128], BF16)
            nc.vector.memset(ones_bf[:], 1.0)
            b2_sb = S.tile([1, 4096], BF16)
            nc.sync.dma_start(out=b2_sb[:], in_=b2r)
            wp = S.tile([B, 4096], FP32)
            for half in range(2):
                pws = []
                for c in range(4):
                    pwc = PW.tile([B, 512], FP32, tag=f"pw{c}", name=f"pw_{half}_{c}")
                    pws.append(pwc)
                for a in range(8):
                    slab = W2P.tile([128, 2048], BF16)
                    nc.sync.dma_start(out=slab[:], in_=w2[half, ts(a, 128), :])
                    for c in range(4):
                        nc.tensor.matmul(
                            pws[c][:],
                            hT[:, ts(a, 128)],
                            slab[:, ts(c, 512)],
                            start=(a == 0),
                            stop=False,
                        )
                for c in range(4):
                    cg = half * 4 + c
                    nc.tensor.matmul(
                        pws[c][:], ones_bf[:], b2_sb[:, ts(cg, 512)],
                        start=False, stop=True,
                    )
                    nc.vector.tensor_copy(wp[:, ts(cg, 512)], pws[c][:])

            if DEBUG:
                nc.sync.dma_start(out=dbg["dbg_wp"], in_=wp[:])
            # ---------------- t2 = sum_d la2*x -> AllReduce -------------
            xcol_sb = S.tile([B, DP], FP32)
            nc.sync.dma_start(out=xcol_sb[:], in_=xcol)
            prod = S.tile([B, 1024], FP32)
            for r in range(R):
                nc.vector.tensor_mul(
                    prod[:, ts(r, 128)], wp[:, ts(r, 128)], xcol_sb[:]
                )
            t2p = SM.tile([B, R], FP32)
            nc.vector.reduce_sum(
                out=t2p[:],
                in_=prod[:].rearrange("p (r d) -> p r d", d=128),
                axis=mybir.AxisListType.X,
            )
            t2in = DR.tile([B, R], FP32)
            t2out = DR.tile([B, R], FP32)
            nc.sync.dma_start(out=t2in[:], in_=t2p[:])
            nc.gpsimd.collective_compute(
                "AllReduce", ALU.add, replica_groups=RG,
                ins=[t2in.opt()], outs=[t2out.opt()],
            )
            t2_sb = SM.tile([B, R], FP32)
            nc.sync.dma_start(out=t2_sb[:], in_=t2out[:])
            if DEBUG:
                nc.sync.dma_start(out=dbg["dbg_t2p"], in_=t2p[:])
                nc.sync.dma_start(out=dbg["dbg_t2"], in_=t2_sb[:])

            # ---------------- y = gelu(x@bd + sum_r lb2*t2) -------------
            xT_sb = S.tile([128, D], FP32)
            nc.sync.dma_start(
                out=xT_sb[:].rearrange("p (t b) -> p t b", t=8),
                in_=xT.rearrange("(t p) b -> p t b", p=128),
            )
            bd_sb = S.tile([128, D], FP32)
            nc.sync.dma_start(
                out=bd_sb[:].rearrange("p (t l) -> p t l", t=8),
                in_=bd.rearrange("(t p) l -> p t l", p=128),
            )
            py0 = PM.tile([B, 512], FP32, tag="ps")
            for dt_ in range(8):
                nc.tensor.matmul(
                    py0[:, 0:128],
                    xT_sb[:, ts(dt_, 128)],
                    bd_sb[:, ts(dt_, 128)],
                    start=(dt_ == 0),
                    stop=(dt_ == 7),
                )
            y_acc = SM.tile([B, DP], FP32)
            nc.vector.tensor_copy(y_acc[:], py0[:, 0:128])
            tmp = SM.tile([B, DP], FP32)
            for r in range(R):
                nc.vector.tensor_scalar_mul(
                    out=tmp[:], in0=wp[:, 1024 + r * 128 : 1024 + (r + 1) * 128],
                    scalar1=t2_sb[:, r : r + 1],
                )
                nc.vector.tensor_add(y_acc[:], y_acc[:], tmp[:])
            y_sb = S.tile([B, DP], FP32)
            nc.scalar.activation(out=y_sb[:], in_=y_acc[:], func=AF.Gelu)

            if DEBUG:
                nc.sync.dma_start(out=dbg["dbg_y"], in_=y_sb[:])
            # ---------------- t1 = sum_l lb1*y -> AllReduce -------------
            prod2 = S.tile([B, 1024], FP32)
            for r in range(R):
                nc.vector.tensor_mul(
                    prod2[:, ts(r, 128)],
                    wp[:, 2048 + r * 128 : 2048 + (r + 1) * 128],
                    y_sb[:],
                )
            t1p = SM.tile([B, R], FP32)
            nc.vector.reduce_sum(
                out=t1p[:],
                in_=prod2[:].rearrange("p (r d) -> p r d", d=128),
                axis=mybir.AxisListType.X,
            )
            t1in = DR.tile([B, R], FP32)
            t1out = DR.tile([B, R], FP32)
            nc.sync.dma_start(out=t1in[:], in_=t1p[:])
            nc.gpsimd.collective_compute(
                "AllReduce", ALU.add, replica_groups=RG,
                ins=[t1in.opt()], outs=[t1out.opt()],
            )
            t1_sb = SM.tile([B, R], FP32)
            nc.sync.dma_start(out=t1_sb[:], in_=t1out[:])

            if DEBUG:
                nc.sync.dma_start(out=dbg["dbg_t1"], in_=t1_sb[:])
            # ---------------- term1 = y_i @ buT -> ReduceScatter --------
            ident_f = S.tile([128, 128], FP32)
            make_identity(nc, ident_f[:])
            pyT = PM.tile([DP, 512], FP32, tag="ps")
            nc.tensor.transpose(pyT[:, 0:128], y_sb[:], ident_f[:])
            yT_sb = SM.tile([DP, B], FP32)
            nc.vector.tensor_copy(yT_sb[:], pyT[:, 0:128])
            buT_sb = S.tile([DP, D], FP32)
            nc.sync.dma_start(out=buT_sb[:], in_=buT)
            rs_sb = S.tile([B, D], FP32)
            pterms = []
            for n in range(2):
                ptn = PM.tile([B, 512], FP32, tag="ps", name=f"pterm{n}")
                pterms.append(ptn)
            for n in range(2):
                nc.tensor.matmul(
                    pterms[n][:], yT_sb[:], buT_sb[:, ts(n, 512)],
                    start=True, stop=False,
                )
            if DEBUG:
                nc.sync.dma_start(out=dbg["dbg_rs"], in_=rs_sb[:])
            # ---------------- term2 = sum_r la1*t1, scattered into rs ----
            term2 = SM.tile([B, DP], FP32)
            tmp2 = SM.tile([B, DP], FP32)
            for r in range(R):
                sl = wp[:, 3072 + r * 128 : 3072 + (r + 1) * 128]
                if r == 0:
                    nc.vector.tensor_scalar_mul(
                        out=term2[:], in0=sl, scalar1=t1_sb[:, r : r + 1]
                    )
                else:
                    nc.vector.tensor_scalar_mul(
                        out=tmp2[:], in0=sl, scalar1=t1_sb[:, r : r + 1]
                    )
                    nc.vector.tensor_add(term2[:], term2[:], tmp2[:])
            if DEBUG:
                nc.sync.dma_start(out=dbg["dbg_term2"], in_=term2[:])
            # scatter term2 into its k-slice of the RS input with a one-hot
            # matmul: pterm[b, k] += sum_kl term2T[kl, b] * eh[kl, k]
            eh_sb = S.tile([DP, D], BF16)
            nc.sync.dma_start(out=eh_sb[:], in_=eh)
            term2_bf = SM.tile([B, DP], BF16)
            nc.vector.tensor_copy(term2_bf[:], term2[:])
            pt2 = PH.tile([128, 128], BF16, tag="pha")
            nc.tensor.transpose(pt2[:], term2_bf[:], ident_bf[:])
            term2T = SM.tile([DP, B], BF16)
            nc.vector.tensor_copy(term2T[:], pt2[:])
            for n in range(2):
                nc.tensor.matmul(
                    pterms[n][:], term2T[:], eh_sb[:, ts(n, 512)],
                    start=False, stop=True,
                )
                nc.vector.tensor_copy(rs_sb[:, ts(n, 512)], pterms[n][:])
            rsin = DR.tile([B, D], FP32)
            rsout = DR.tile([BP, D], FP32)
            nc.sync.dma_start(out=rsin[:], in_=rs_sb[:])
            nc.gpsimd.collective_compute(
                "ReduceScatter", ALU.add, replica_groups=RG,
                ins=[rsin.opt()], outs=[rsout.opt()],
            )

            # ---------------- final: out = x + rsout --------------------
            fin = SM.tile([BP, D], FP32)
            nc.sync.dma_start(out=fin[:], in_=rsout[:])
            xres_sb = SM.tile([BP, D], FP32)
            nc.sync.dma_start(out=xres_sb[:], in_=xres)
            nc.vector.tensor_add(fin[:], fin[:], xres_sb[:])
            nc.sync.dma_start(out=outp, in_=fin[:])

    _split_excess_waits(nc)
    return nc


_NC = None


def _get_nc():
    global _NC
    if _NC is None:
        _NC = build_graph()
    return _NC


def make_in_maps(x, ada_emb, base_up, base_down, ln_g, ln_b, W1, b1, W2, b2):
    f32 = np.float32
    bf16 = ml_dtypes.bfloat16
    x = np.asarray(x, f32)
    ada_emb = np.ascontiguousarray(np.asarray(ada_emb, f32))
    W1 = np.asarray(W1, f32)
    W2 = np.asarray(W2, f32)
    # fold LN affine into W1/b1
    W1p = (np.asarray(ln_g, f32)[:, None] * W1).astype(bf16)
    b1p = (np.asarray(b1, f32) + np.asarray(ln_b, f32) @ W1).astype(f32)
    b1c = np.ascontiguousarray(b1p.reshape(8, 128).T)
    xT = np.ascontiguousarray(x.T)
    W2r = W2.reshape(ADA, 4, D, R)
    b2r_full = np.asarray(b2, f32).reshape(4, D, R)
    chunk_order = [2, 3, 1, 0]  # la2, lb2, lb1, la1

    in_maps = []
    for i in range(NCORES):
        dl = slice(i * DP, (i + 1) * DP)
        sl = W2r[:, chunk_order, dl, :]            # [a, 4, 128, 8]
        sl = sl.transpose(1, 3, 2, 0)              # [4, r, dl, a]
        # cols (chunk, r, dl): slab[a, chunk*1024 + r*128 + dl]
        slab = np.ascontiguousarray(sl.reshape(4096, ADA).T)   # [a, 4096]
        w2i = np.ascontiguousarray(
            slab.reshape(ADA, 2, 2048).transpose(1, 0, 2)
        ).astype(bf16)                              # [2, a, 2048]
        b2sl = b2r_full[chunk_order, dl, :]         # [4, 128, 8]
        b2i = np.ascontiguousarray(
            b2sl.transpose(0, 2, 1).reshape(1, 4096)
        ).astype(bf16)
        in_maps.append(
            {
                "ada": ada_emb,
                "w1": W1p,
                "b1c": b1c,
                "w2": w2i,
                "b2r": b2i,
                "xT": xT,
                "xcol": np.ascontiguousarray(x[:, dl]),
                "bd": np.ascontiguousarray(np.asarray(base_down, f32)[:, dl]),
                "buT": np.ascontiguousarray(np.asarray(base_up, f32)[:, dl].T),
                "xres": np.ascontiguousarray(x[i * BP : (i + 1) * BP, :]),
                "eh": np.ascontiguousarray(
                    np.eye(D, dtype=f32)[i * DP : (i + 1) * DP, :]
                ).astype(bf16),
            }
        )
    return in_maps


def run(trace=False, tmpdir=None, **inputs):
    nc = _get_nc()
    in_maps = make_in_maps(**inputs)
    res = bass_utils.run_bass_kernel_spmd(
        nc, in_maps, core_ids=list(range(NCORES)), trace=trace, tmpdir=tmpdir
    )
    out = np.concatenate([res.results[i]["out"] for i in range(NCORES)], axis=0)
    return out, res


def kernel(**inputs):
    out, _ = run(trace=False, **inputs)
    return out


# revision 12
# speedup vs baseline: 1.1352x; 1.0573x over previous
"""AdaLoRA MLP with base — 8-core Trainium2 Bass kernel.

Strategy (tensor-parallel W2 + d-sharded LoRA factors):
  - The dominant cost is reading W2 [1024, 32768] f32 (134 MB). Each core owns
    a 1/8 column slice (respecting the 4 chunk boundaries la1|lb1|la2|lb2),
    stored bf16, cg-major, and computes w_part = h @ W2_i for ALL 128 samples.
  - Per-sample LoRA factors then live d-sharded: rank i holds la*/lb*[b, d, r]
    for d in Di = [128*i, 128*(i+1)). All contractions over d become rank-local
    partial sums + tiny collectives:
      t2 = sum_d la2*x   -> AllReduce [128,8]
      y  = x@bd[:,Di] + sum_r lb2*t2 (gelu)        (column-local, no comm)
      t1 = sum_l lb1*y   -> AllReduce [128,8]
      term1 = y_i @ base_up[:,Di].T (partial sum)  -> ReduceScatter [128,1024]
      term2 = sum_r la1*t1 (k-local)               -> AllToAll [128,128]
  - Output: rank i returns out[16*i:16*(i+1), :]; host concatenates.

h-path: LayerNorm on-chip (g/beta folded into W1/b1 on host), h computed
directly transposed (hT[a,b]) in bf16 so it feeds the W2 matmul as stationary.
"""

import sys

sys.path.insert(0, "/opt/trn_rl_repo")

import numpy as np
import ml_dtypes

import concourse.bass as bass
import concourse.tile as tile
from concourse import mybir, bass_utils
from concourse.bass import ts
from concourse.masks import make_identity
from concourse.vector_clock import ScopedClock

B, D, ADA, R = 128, 1024, 1024, 8
NCORES = 8
BP = B // NCORES      # samples per core = 16
DP = D // NCORES      # d-slice per core = 128
FP32 = mybir.dt.float32
BF16 = mybir.dt.bfloat16
AF = mybir.ActivationFunctionType
ALU = mybir.AluOpType

# walrus in this environment rejects InstDrain carrying sem-eq waits or >1
# wait command; route barrier waits through EventSemaphores and split the
# tail drain's waits into individual wait_ge instructions.
_PATCHED = False


def _apply_patches():
    global _PATCHED
    if _PATCHED:
        return
    _PATCHED = True

    def multi_engine_barrier(self, engines):
        for e in engines:
            self.engines[e].drain()
        tag = f"mbar{self.next_id()}"
        for inst in self._sem_only_all_engine_barrier_insts(tag):
            self.engines[inst.engine].add_instruction(inst)

    bass.Bass.multi_engine_barrier = multi_engine_barrier

    def _drain_and_barrier(self, tick_clock, wait_clock):
        nc = self.nc
        drain_inst = nc.sync.drain()
        wait_clock.add_sem_waits(
            drain_inst.ins, ScopedClock({None: tick_clock.global_clock})
        )
        si = drain_inst.ins.sync_info
        waits = list(si.on_wait or []) if si else []
        if len(waits) > 1:
            drain_inst.ins.sync_info = mybir.SyncInfo(
                on_wait=[waits[0]], on_update=list(si.on_update or [])
            )
            handle_map = {h.num: h for h in self.sems.allocated().values()}
            for w in waits[1:]:
                h = handle_map.get(w.id)
                assert h is not None and w.wait_mode == "sem-ge-imm"
                nc.sync.wait_ge(h, w.wait_value)
        nc.all_engine_barrier()
        popped = nc._tile_sem_poison_stack.pop()
        assert popped is self._sem_poison
        nc.clear_and_free_semaphores(list(self.sems.allocated().values()))

    tile.TileContext._drain_and_barrier = _drain_and_barrier


def _split_excess_waits(nc, max_waits=1):
    """walrus here encodes at most ~1 sync-wait command per instruction;
    hoist excess waits onto NoOp carriers placed just before (same engine)."""
    for fn in nc.m.functions:
        for bb in fn.blocks:
            new = []
            changed = False
            for ins in bb.instructions:
                si = ins.sync_info
                waits = list(si.on_wait or []) if si else []
                if len(waits) > max_waits:
                    changed = True
                    for w in waits[:-max_waits]:
                        nop = mybir.InstNoOp(name=f"I-wsplit-{nc.next_id()}")
                        nop.engine = ins.engine
                        nop.sync_info = mybir.SyncInfo(on_wait=[w], on_update=[])
                        nc.register_instruction(nop, overwrite=True)
                        new.append(nop)
                    ins.sync_info = mybir.SyncInfo(
                        on_wait=waits[-max_waits:], on_update=list(si.on_update or [])
                    )
                new.append(ins)
            if changed:
                bb.instructions = new


DEBUG = False


def build_graph():
    _apply_patches()
    nc = bass.Bass("TRN2", target_bir_lowering=False, debug=False, num_devices=NCORES)
    RG = [list(range(NCORES))]

    ada = nc.dram_tensor("ada", [B, ADA], FP32, kind="ExternalInput").ap()
    w1 = nc.dram_tensor("w1", [ADA, ADA], BF16, kind="ExternalInput").ap()
    b1c = nc.dram_tensor("b1c", [128, 8], FP32, kind="ExternalInput").ap()
    w2 = nc.dram_tensor("w2", [2, ADA, 2048], BF16, kind="ExternalInput").ap()
    b2r = nc.dram_tensor("b2r", [1, 4096], BF16, kind="ExternalInput").ap()
    xT = nc.dram_tensor("xT", [D, B], FP32, kind="ExternalInput").ap()
    xcol = nc.dram_tensor("xcol", [B, DP], FP32, kind="ExternalInput").ap()
    bd = nc.dram_tensor("bd", [D, DP], FP32, kind="ExternalInput").ap()
    buT = nc.dram_tensor("buT", [DP, D], FP32, kind="ExternalInput").ap()
    xres = nc.dram_tensor("xres", [BP, D], FP32, kind="ExternalInput").ap()
    eh = nc.dram_tensor("eh", [DP, D], BF16, kind="ExternalInput").ap()
    outp = nc.dram_tensor("out", [BP, D], FP32, kind="ExternalOutput").ap()
    dbg = {}
    if DEBUG:
        for nm, shp in [("dbg_c", [B, ADA]), ("dbg_hT", [128, ADA]),
                        ("dbg_wp", [B, 4096]), ("dbg_t2p", [B, R]),
                        ("dbg_t2", [B, R]), ("dbg_y", [B, DP]),
                        ("dbg_t1", [B, R]), ("dbg_rs", [B, D]),
                        ("dbg_term2", [B, DP]), ("dbg_rsout", [BP, D]),
                        ("dbg_a2a", [BP, D])]:
            dbg[nm] = nc.dram_tensor(nm, shp, FP32, kind="ExternalOutput").ap()

    with tile.TileContext(nc) as tc:
        with (
            tc.tile_pool(name="singles", bufs=1) as S,
            tc.tile_pool(name="small", bufs=2) as SM,
            tc.tile_pool(name="w1pool", bufs=1) as W1P,
            tc.tile_pool(name="w2pool", bufs=6) as W2P,
            tc.tile_pool(name="ph", bufs=2, space="PSUM") as PH,
            tc.tile_pool(name="pw", bufs=1, space="PSUM") as PW,
            tc.tile_pool(name="pmisc", bufs=2, space="PSUM") as PM,
            tc.tile_pool(name="dram", bufs=1, space="DRAM") as DR,
        ):
            # ---------------- Phase A: LayerNorm + transpose c ----------
            ada_sb = S.tile([B, ADA], FP32)
            nc.sync.dma_start(out=ada_sb[:], in_=ada)

            eps = S.tile([B, 1], FP32)
            nc.vector.memset(eps[:], 1e-5)
            stats = SM.tile([B, 2, nc.vector.BN_STATS_DIM], FP32)
            for sg in range(2):
                nc.vector.bn_stats(
                    out=stats[:, sg, :], in_=ada_sb[:, ts(sg, 512)]
                )
            mv = SM.tile([B, nc.vector.BN_AGGR_DIM], FP32)
            nc.vector.bn_aggr(out=mv[:], in_=stats[:])
            rstd = SM.tile([B, 1], FP32)
            nc.scalar.activation(
                out=rstd[:], in_=mv[:, 1:2], func=AF.Sqrt, bias=eps[:], scale=1.0
            )
            nc.vector.reciprocal(out=rstd[:], in_=rstd[:])
            c_bf = S.tile([B, ADA], BF16)
            nc.vector.tensor_scalar(
                out=c_bf[:],
                in0=ada_sb[:],
                scalar1=mv[:, 0:1],
                scalar2=rstd[:],
                op0=ALU.subtract,
                op1=ALU.mult,
            )

            if DEBUG:
                cdump = SM.tile([B, ADA], FP32)
                nc.vector.tensor_copy(cdump[:], c_bf[:])
                nc.sync.dma_start(out=dbg["dbg_c"], in_=cdump[:])
            ident_bf = S.tile([128, 128], BF16)
            make_identity(nc, ident_bf[:])
            cT = S.tile([128, ADA], BF16)  # 8 tiles of c.T
            for e in range(8):
                p = PM.tile([128, 512], BF16, tag="ps")
                nc.tensor.transpose(p[:, 0:128], c_bf[:, ts(e, 128)], ident_bf[:])
                nc.vector.tensor_copy(cT[:, ts(e, 128)], p[:, 0:128])

            # ---------------- Phase B: hT = gelu(W1'.T @ c.T + b1') -----
            b1c_sb = S.tile([128, 8], FP32)
            nc.sync.dma_start(out=b1c_sb[:], in_=b1c)
            # one single-bank PSUM accumulation group per a-tile: start=True
            # clears the whole bank, so groups must not interleave in a bank
            w1rows = []
            for e in range(8):
                w1row = W1P.tile([128, ADA], BF16, tag=f"w1r{e}")
                nc.sync.dma_start(out=w1row[:], in_=w1[ts(e, 128), :])
                w1rows.append(w1row)
            hT = S.tile([128, ADA], BF16)
            for a in range(8):
                pha = PH.tile([128, 128], FP32, tag="pha")
                for e in range(8):
                    nc.tensor.matmul(
                        pha[:],
                        w1rows[e][:, ts(a, 128)],
                        cT[:, ts(e, 128)],
                        start=(e == 0),
                        stop=(e == 7),
                    )
                nc.scalar.activation(
                    out=hT[:, ts(a, 128)],
                    in_=pha[:],
                    func=AF.Gelu,
                    bias=b1c_sb[:, a : a + 1],
                    scale=1.0,
                )

            if DEBUG:
                hdump = SM.tile([128, ADA], FP32)
                nc.vector.tensor_copy(hdump[:], hT[:])
                nc.sync.dma_start(out=dbg["dbg_hT"], in_=hdump[:])
            # ---------------- Phase C: w_part = h @ W2_i + b2_i ---------
            # slab cols: [la2 | lb2 | lb1 | la1], within chunk col = r*128+dl
            ones_bf = S.tile([1, 128], BF16)
            nc.vector.memset(ones_bf[:], 1.0)
            b2_sb = S.tile([1, 4096], BF16)
            nc.sync.dma_start(out=b2_sb[:], in_=b2r)
            wp = S.tile([B, 4096], FP32)
            for half in range(2):
                pws = []
                for c in range(4):
                    pwc = PW.tile([B, 512], FP32, tag=f"pw{c}", name=f"pw_{half}_{c}")
                    pws.append(pwc)
                for a in range(8):
                    slab = W2P.tile([128, 2048], BF16)
                    nc.sync.dma_start(out=slab[:], in_=w2[half, ts(a, 128), :])
                    for c in range(4):
                        nc.tensor.matmul(
                            pws[c][:],
                            hT[:, ts(a, 128)],
                            slab[:, ts(c, 512)],
                            start=(a == 0),
                            stop=False,
                        )
                for c in range(4):
                    cg = half * 4 + c
                    nc.tensor.matmul(
                        pws[c][:], ones_bf[:], b2_sb[:, ts(cg, 512)],
                        start=False, stop=True,
                    )
                    nc.vector.tensor_copy(wp[:, ts(cg, 512)], pws[c][:])

            if DEBUG:
                nc.sync.dma_start(out=dbg["dbg_wp"], in_=wp[:])
            # ---------------- t2 = sum_d la2*x -> AllReduce -------------
            xcol_sb = S.tile([B, DP], FP32)
            nc.sync.dma_start(out=xcol_sb[:], in_=xcol)
            prod = S.tile([B, DP], FP32)
            t2p = SM.tile([B, R], FP32)
            for r in range(R):
                nc.vector.scalar_tensor_tensor(
                    out=prod[:], in0=wp[:, ts(r, 128)], scalar=1.0,
                    in1=xcol_sb[:], op0=ALU.bypass, op1=ALU.mult,
                    accum_out=t2p[:, r : r + 1],
                )
            t2in = DR.tile([B, R], FP32)
            t2out = DR.tile([B, R], FP32)
            nc.sync.dma_start(out=t2in[:], in_=t2p[:])
            nc.gpsimd.collective_compute(
                "AllReduce", ALU.add, replica_groups=RG,
                ins=[t2in.opt()], outs=[t2out.opt()],
            )
            t2_sb = SM.tile([B, R], FP32)
            nc.sync.dma_start(out=t2_sb[:], in_=t2out[:])
            if DEBUG:
                nc.sync.dma_start(out=dbg["dbg_t2p"], in_=t2p[:])
                nc.sync.dma_start(out=dbg["dbg_t2"], in_=t2_sb[:])

            # ---------------- y = gelu(x@bd + sum_r lb2*t2) -------------
            xT_sb = S.tile([128, D], FP32)
            nc.sync.dma_start(
                out=xT_sb[:].rearrange("p (t b) -> p t b", t=8),
                in_=xT.rearrange("(t p) b -> p t b", p=128),
            )
            bd_sb = S.tile([128, D], FP32)
            nc.sync.dma_start(
                out=bd_sb[:].rearrange("p (t l) -> p t l", t=8),
                in_=bd.rearrange("(t p) l -> p t l", p=128),
            )
            py0 = PM.tile([B, 512], FP32, tag="ps")
            for dt_ in range(8):
                nc.tensor.matmul(
                    py0[:, 0:128],
                    xT_sb[:, ts(dt_, 128)],
                    bd_sb[:, ts(dt_, 128)],
                    start=(dt_ == 0),
                    stop=(dt_ == 7),
                )
            y_acc = SM.tile([B, DP], FP32)
            nc.vector.tensor_copy(y_acc[:], py0[:, 0:128])
            for r in range(R):
                nc.vector.scalar_tensor_tensor(
                    out=y_acc[:], in0=wp[:, 1024 + r * 128 : 1024 + (r + 1) * 128],
                    scalar=t2_sb[:, r : r + 1], in1=y_acc[:],
                    op0=ALU.mult, op1=ALU.add,
                )
            y_sb = S.tile([B, DP], FP32)
            nc.scalar.activation(out=y_sb[:], in_=y_acc[:], func=AF.Gelu)

            if DEBUG:
                nc.sync.dma_start(out=dbg["dbg_y"], in_=y_sb[:])
            # ---------------- t1 = sum_l lb1*y -> AllReduce -------------
            prod2 = S.tile([B, DP], FP32)
            t1p = SM.tile([B, R], FP32)
            for r in range(R):
                nc.vector.scalar_tensor_tensor(
                    out=prod2[:], in0=wp[:, 2048 + r * 128 : 2048 + (r + 1) * 128],
                    scalar=1.0, in1=y_sb[:], op0=ALU.bypass, op1=ALU.mult,
                    accum_out=t1p[:, r : r + 1],
                )
            t1in = DR.tile([B, R], FP32)
            t1out = DR.tile([B, R], FP32)
            nc.sync.dma_start(out=t1in[:], in_=t1p[:])
            nc.gpsimd.collective_compute(
                "AllReduce", ALU.add, replica_groups=RG,
                ins=[t1in.opt()], outs=[t1out.opt()],
            )
            t1_sb = SM.tile([B, R], FP32)
            nc.sync.dma_start(out=t1_sb[:], in_=t1out[:])

            if DEBUG:
                nc.sync.dma_start(out=dbg["dbg_t1"], in_=t1_sb[:])
            # ---------------- term1 = y_i @ buT -> ReduceScatter --------
            ident_f = S.tile([128, 128], FP32)
            make_identity(nc, ident_f[:])
            pyT = PM.tile([DP, 512], FP32, tag="ps")
            nc.tensor.transpose(pyT[:, 0:128], y_sb[:], ident_f[:])
            yT_sb = SM.tile([DP, B], FP32)
            nc.vector.tensor_copy(yT_sb[:], pyT[:, 0:128])
            buT_sb = S.tile([DP, D], FP32)
            nc.sync.dma_start(out=buT_sb[:], in_=buT)
            rs_sb = S.tile([B, D], BF16)
            pterms = []
            for n in range(2):
                ptn = PM.tile([B, 512], FP32, tag="ps", name=f"pterm{n}")
                pterms.append(ptn)
            for n in range(2):
                nc.tensor.matmul(
                    pterms[n][:], yT_sb[:], buT_sb[:, ts(n, 512)],
                    start=True, stop=False,
                )
            if DEBUG:
                nc.sync.dma_start(out=dbg["dbg_rs"], in_=rs_sb[:])
            # ---------------- term2 = sum_r la1*t1, scattered into rs ----
            term2 = SM.tile([B, DP], FP32)
            for r in range(R):
                sl = wp[:, 3072 + r * 128 : 3072 + (r + 1) * 128]
                if r == 0:
                    nc.vector.tensor_scalar_mul(
                        out=term2[:], in0=sl, scalar1=t1_sb[:, r : r + 1]
                    )
                else:
                    nc.vector.scalar_tensor_tensor(
                        out=term2[:], in0=sl, scalar=t1_sb[:, r : r + 1],
                        in1=term2[:], op0=ALU.mult, op1=ALU.add,
                    )
            if DEBUG:
                nc.sync.dma_start(out=dbg["dbg_term2"], in_=term2[:])
            # scatter term2 into its k-slice of the RS input with a one-hot
            # matmul: pterm[b, k] += sum_kl term2T[kl, b] * eh[kl, k]
            eh_sb = S.tile([DP, D], BF16)
            nc.sync.dma_start(out=eh_sb[:], in_=eh)
            term2_bf = SM.tile([B, DP], BF16)
            nc.vector.tensor_copy(term2_bf[:], term2[:])
            pt2 = PH.tile([128, 128], BF16, tag="pha")
            nc.tensor.transpose(pt2[:], term2_bf[:], ident_bf[:])
            term2T = SM.tile([DP, B], BF16)
            nc.vector.tensor_copy(term2T[:], pt2[:])
            for n in range(2):
                nc.tensor.matmul(
                    pterms[n][:], term2T[:], eh_sb[:, ts(n, 512)],
                    start=False, stop=True,
                )
                nc.vector.tensor_copy(rs_sb[:, ts(n, 512)], pterms[n][:])
            rsin = DR.tile([B, D], BF16)
            rsout = DR.tile([BP, D], BF16)
            nc.sync.dma_start(out=rsin[:], in_=rs_sb[:])
            nc.gpsimd.collective_compute(
                "ReduceScatter", ALU.add, replica_groups=RG,
                ins=[rsin.opt()], outs=[rsout.opt()],
            )

            # ---------------- final: out = x + rsout --------------------
            fin_bf = SM.tile([BP, D], BF16)
            nc.sync.dma_start(out=fin_bf[:], in_=rsout[:])
            xres_sb = SM.tile([BP, D], FP32)
            nc.sync.dma_start(out=xres_sb[:], in_=xres)
            fin = SM.tile([BP, D], FP32)
            nc.vector.tensor_add(fin[:], fin_bf[:], xres_sb[:])
            nc.sync.dma_start(out=outp, in_=fin[:])

    _split_excess_waits(nc)
    return nc


_NC = None


def _get_nc():
    global _NC
    if _NC is None:
        _NC = build_graph()
    return _NC


def make_in_maps(x, ada_emb, base_up, base_down, ln_g, ln_b, W1, b1, W2, b2):
    f32 = np.float32
    bf16 = ml_dtypes.bfloat16
    x = np.asarray(x, f32)
    ada_emb = np.ascontiguousarray(np.asarray(ada_emb, f32))
    W1 = np.asarray(W1, f32)
    W2 = np.asarray(W2, f32)
    # fold LN affine into W1/b1
    W1p = (np.asarray(ln_g, f32)[:, None] * W1).astype(bf16)
    b1p = (np.asarray(b1, f32) + np.asarray(ln_b, f32) @ W1).astype(f32)
    b1c = np.ascontiguousarray(b1p.reshape(8, 128).T)
    xT = np.ascontiguousarray(x.T)
    W2r = W2.reshape(ADA, 4, D, R)
    b2r_full = np.asarray(b2, f32).reshape(4, D, R)
    chunk_order = [2, 3, 1, 0]  # la2, lb2, lb1, la1

    in_maps = []
    for i in range(NCORES):
        dl = slice(i * DP, (i + 1) * DP)
        sl = W2r[:, chunk_order, dl, :]            # [a, 4, 128, 8]
        sl = sl.transpose(1, 3, 2, 0)              # [4, r, dl, a]
        # cols (chunk, r, dl): slab[a, chunk*1024 + r*128 + dl]
        slab = np.ascontiguousarray(sl.reshape(4096, ADA).T)   # [a, 4096]
        w2i = np.ascontiguousarray(
            slab.reshape(ADA, 2, 2048).transpose(1, 0, 2)
        ).astype(bf16)                              # [2, a, 2048]
        b2sl = b2r_full[chunk_order, dl, :]         # [4, 128, 8]
        b2i = np.ascontiguousarray(
            b2sl.transpose(0, 2, 1).reshape(1, 4096)
        ).astype(bf16)
        in_maps.append(
            {
                "ada": ada_emb,
                "w1": W1p,
                "b1c": b1c,
                "w2": w2i,
                "b2r": b2i,
                "xT": xT,
                "xcol": np.ascontiguousarray(x[:, dl]),
                "bd": np.ascontiguousarray(np.asarray(base_down, f32)[:, dl]),
                "buT": np.ascontiguousarray(np.asarray(base_up, f32)[:, dl].T),
                "xres": np.ascontiguousarray(x[i * BP : (i + 1) * BP, :]),
                "eh": np.ascontiguousarray(
                    np.eye(D, dtype=f32)[i * DP : (i + 1) * DP, :]
                ).astype(bf16),
            }
        )
    return in_maps


def run(trace=False, tmpdir=None, **inputs):
    nc = _get_nc()
    in_maps = make_in_maps(**inputs)
    res = bass_utils.run_bass_kernel_spmd(
        nc, in_maps, core_ids=list(range(NCORES)), trace=trace, tmpdir=tmpdir
    )
    out = np.concatenate([res.results[i]["out"] for i in range(NCORES)], axis=0)
    return out, res


def kernel(**inputs):
    out, _ = run(trace=False, **inputs)
    return out


# revision 14
# speedup vs baseline: 1.1560x; 1.0183x over previous
"""AdaLoRA MLP with base — 8-core Trainium2 Bass kernel.

Strategy (tensor-parallel W2 + d-sharded LoRA factors):
  - The dominant cost is reading W2 [1024, 32768] f32 (134 MB). Each core owns
    a 1/8 column slice (respecting the 4 chunk boundaries la1|lb1|la2|lb2),
    stored bf16, cg-major, and computes w_part = h @ W2_i for ALL 128 samples.
  - Per-sample LoRA factors then live d-sharded: rank i holds la*/lb*[b, d, r]
    for d in Di = [128*i, 128*(i+1)). All contractions over d become rank-local
    partial sums + tiny collectives:
      t2 = sum_d la2*x   -> AllReduce [128,8]
      y  = x@bd[:,Di] + sum_r lb2*t2 (gelu)        (column-local, no comm)
      t1 = sum_l lb1*y   -> AllReduce [128,8]
      term1 = y_i @ base_up[:,Di].T (partial sum)  -> ReduceScatter [128,1024]
      term2 = sum_r la1*t1 (k-local)               -> AllToAll [128,128]
  - Output: rank i returns out[16*i:16*(i+1), :]; host concatenates.

h-path: LayerNorm on-chip (g/beta folded into W1/b1 on host), h computed
directly transposed (hT[a,b]) in bf16 so it feeds the W2 matmul as stationary.
"""

import sys

sys.path.insert(0, "/opt/trn_rl_repo")

import numpy as np
import ml_dtypes

import concourse.bass as bass
import concourse.tile as tile
from concourse import mybir, bass_utils
from concourse.bass import ts
from concourse.masks import make_identity
from concourse.vector_clock import ScopedClock

B, D, ADA, R = 128, 1024, 1024, 8
NCORES = 8
BP = B // NCORES      # samples per core = 16
DP = D // NCORES      # d-slice per core = 128
FP32 = mybir.dt.float32
BF16 = mybir.dt.bfloat16
AF = mybir.ActivationFunctionType
ALU = mybir.AluOpType

# walrus in this environment rejects InstDrain carrying sem-eq waits or >1
# wait command; route barrier waits through EventSemaphores and split the
# tail drain's waits into individual wait_ge instructions.
_PATCHED = False


def _apply_patches():
    global _PATCHED
    if _PATCHED:
        return
    _PATCHED = True

    def multi_engine_barrier(self, engines):
        for e in engines:
            self.engines[e].drain()
        tag = f"mbar{self.next_id()}"
        for inst in self._sem_only_all_engine_barrier_insts(tag):
            self.engines[inst.engine].add_instruction(inst)

    bass.Bass.multi_engine_barrier = multi_engine_barrier

    def _drain_and_barrier(self, tick_clock, wait_clock):
        nc = self.nc
        drain_inst = nc.sync.drain()
        wait_clock.add_sem_waits(
            drain_inst.ins, ScopedClock({None: tick_clock.global_clock})
        )
        si = drain_inst.ins.sync_info
        waits = list(si.on_wait or []) if si else []
        if len(waits) > 1:
            drain_inst.ins.sync_info = mybir.SyncInfo(
                on_wait=[waits[0]], on_update=list(si.on_update or [])
            )
            handle_map = {h.num: h for h in self.sems.allocated().values()}
            for w in waits[1:]:
                h = handle_map.get(w.id)
                assert h is not None and w.wait_mode == "sem-ge-imm"
                nc.sync.wait_ge(h, w.wait_value)
        nc.all_engine_barrier()
        popped = nc._tile_sem_poison_stack.pop()
        assert popped is self._sem_poison
        nc.clear_and_free_semaphores(list(self.sems.allocated().values()))

    tile.TileContext._drain_and_barrier = _drain_and_barrier


def _split_excess_waits(nc, max_waits=1):
    """walrus here encodes at most ~1 sync-wait command per instruction;
    hoist excess waits onto NoOp carriers placed just before (same engine)."""
    for fn in nc.m.functions:
        for bb in fn.blocks:
            new = []
            changed = False
            for ins in bb.instructions:
                si = ins.sync_info
                waits = list(si.on_wait or []) if si else []
                if len(waits) > max_waits:
                    changed = True
                    for w in waits[:-max_waits]:
                        nop = mybir.InstNoOp(name=f"I-wsplit-{nc.next_id()}")
                        nop.engine = ins.engine
                        nop.sync_info = mybir.SyncInfo(on_wait=[w], on_update=[])
                        nc.register_instruction(nop, overwrite=True)
                        new.append(nop)
                    ins.sync_info = mybir.SyncInfo(
                        on_wait=waits[-max_waits:], on_update=list(si.on_update or [])
                    )
                new.append(ins)
            if changed:
                bb.instructions = new


DEBUG = False


def build_graph():
    _apply_patches()
    nc = bass.Bass("TRN2", target_bir_lowering=False, debug=False, num_devices=NCORES)
    RG = [list(range(NCORES))]

    ada = nc.dram_tensor("ada", [B, ADA], FP32, kind="ExternalInput").ap()
    w1 = nc.dram_tensor("w1", [ADA, ADA], BF16, kind="ExternalInput").ap()
    b1c = nc.dram_tensor("b1c", [128, 8], FP32, kind="ExternalInput").ap()
    w2 = nc.dram_tensor("w2", [2, ADA, 2048], BF16, kind="ExternalInput").ap()
    b2r = nc.dram_tensor("b2r", [1, 4096], BF16, kind="ExternalInput").ap()
    xT = nc.dram_tensor("xT", [D, B], FP32, kind="ExternalInput").ap()
    xcol = nc.dram_tensor("xcol", [B, DP], FP32, kind="ExternalInput").ap()
    bd = nc.dram_tensor("bd", [D, DP], FP32, kind="ExternalInput").ap()
    buT = nc.dram_tensor("buT", [DP, D], FP32, kind="ExternalInput").ap()
    xres = nc.dram_tensor("xres", [BP, D], FP32, kind="ExternalInput").ap()
    outp = nc.dram_tensor("out", [BP, D], FP32, kind="ExternalOutput").ap()
    dbg = {}
    if DEBUG:
        for nm, shp in [("dbg_c", [B, ADA]), ("dbg_hT", [128, ADA]),
                        ("dbg_wp", [B, 4096]), ("dbg_t2p", [B, R]),
                        ("dbg_t2", [B, R]), ("dbg_y", [B, DP]),
                        ("dbg_t1", [B, R]), ("dbg_rs", [B, D]),
                        ("dbg_term2", [B, DP]), ("dbg_rsout", [BP, D]),
                        ("dbg_a2a", [BP, D])]:
            dbg[nm] = nc.dram_tensor(nm, shp, FP32, kind="ExternalOutput").ap()

    with tile.TileContext(nc) as tc:
        with (
            tc.tile_pool(name="singles", bufs=1) as S,
            tc.tile_pool(name="small", bufs=2) as SM,
            tc.tile_pool(name="w1pool", bufs=1) as W1P,
            tc.tile_pool(name="w2pool", bufs=6) as W2P,
            tc.tile_pool(name="ph", bufs=2, space="PSUM") as PH,
            tc.tile_pool(name="pw", bufs=1, space="PSUM") as PW,
            tc.tile_pool(name="pmisc", bufs=2, space="PSUM") as PM,
            tc.tile_pool(name="dram", bufs=1, space="DRAM") as DR,
        ):
            # ---------------- small params first (unblock dependents) ---
            ada_sb = S.tile([B, ADA], FP32)
            nc.sync.dma_start(out=ada_sb[:], in_=ada)
            b1c_sb = S.tile([128, 8], FP32)
            nc.sync.dma_start(out=b1c_sb[:], in_=b1c)
            b2_sb = S.tile([1, 4096], BF16)
            nc.sync.dma_start(out=b2_sb[:], in_=b2r)
            xcol_sb = S.tile([B, DP], FP32)
            nc.sync.dma_start(out=xcol_sb[:], in_=xcol)
            xT_sb = S.tile([128, D], FP32)
            nc.sync.dma_start(
                out=xT_sb[:].rearrange("p (t b) -> p t b", t=8),
                in_=xT.rearrange("(t p) b -> p t b", p=128),
            )
            bd_sb = S.tile([128, D], FP32)
            nc.sync.dma_start(
                out=bd_sb[:].rearrange("p (t l) -> p t l", t=8),
                in_=bd.rearrange("(t p) l -> p t l", p=128),
            )
            buT_sb = S.tile([DP, D], FP32)
            nc.sync.dma_start(out=buT_sb[:], in_=buT)
            xres_sb = SM.tile([BP, D], FP32)
            nc.sync.dma_start(out=xres_sb[:], in_=xres)
            # ---------------- Phase A: LayerNorm + transpose c ----------

            eps = S.tile([B, 1], FP32)
            nc.vector.memset(eps[:], 1e-5)
            stats = SM.tile([B, 2, nc.vector.BN_STATS_DIM], FP32)
            for sg in range(2):
                nc.vector.bn_stats(
                    out=stats[:, sg, :], in_=ada_sb[:, ts(sg, 512)]
                )
            mv = SM.tile([B, nc.vector.BN_AGGR_DIM], FP32)
            nc.vector.bn_aggr(out=mv[:], in_=stats[:])
            rstd = SM.tile([B, 1], FP32)
            nc.scalar.activation(
                out=rstd[:], in_=mv[:, 1:2], func=AF.Sqrt, bias=eps[:], scale=1.0
            )
            nc.vector.reciprocal(out=rstd[:], in_=rstd[:])
            c_bf = S.tile([B, ADA], BF16)
            nc.vector.tensor_scalar(
                out=c_bf[:],
                in0=ada_sb[:],
                scalar1=mv[:, 0:1],
                scalar2=rstd[:],
                op0=ALU.subtract,
                op1=ALU.mult,
            )

            if DEBUG:
                cdump = SM.tile([B, ADA], FP32)
                nc.vector.tensor_copy(cdump[:], c_bf[:])
                nc.sync.dma_start(out=dbg["dbg_c"], in_=cdump[:])
            ident_bf = S.tile([128, 128], BF16)
            make_identity(nc, ident_bf[:])
            cT = S.tile([128, ADA], BF16)  # 8 tiles of c.T
            for e in range(8):
                p = PM.tile([128, 512], BF16, tag="ps")
                nc.tensor.transpose(p[:, 0:128], c_bf[:, ts(e, 128)], ident_bf[:])
                nc.vector.tensor_copy(cT[:, ts(e, 128)], p[:, 0:128])

            # ---------------- Phase B: hT = gelu(W1'.T @ c.T + b1') -----
            # one single-bank PSUM accumulation group per a-tile: start=True
            # clears the whole bank, so groups must not interleave in a bank
            w1rows = []
            for e in range(8):
                w1row = W1P.tile([128, ADA], BF16, tag=f"w1r{e}")
                nc.sync.dma_start(out=w1row[:], in_=w1[ts(e, 128), :])
                w1rows.append(w1row)
            hT = S.tile([128, ADA], BF16)
            for a in range(8):
                pha = PH.tile([128, 128], FP32, tag="pha")
                for e in range(8):
                    nc.tensor.matmul(
                        pha[:],
                        w1rows[e][:, ts(a, 128)],
                        cT[:, ts(e, 128)],
                        start=(e == 0),
                        stop=(e == 7),
                    )
                nc.scalar.activation(
                    out=hT[:, ts(a, 128)],
                    in_=pha[:],
                    func=AF.Gelu,
                    bias=b1c_sb[:, a : a + 1],
                    scale=1.0,
                )

            if DEBUG:
                hdump = SM.tile([128, ADA], FP32)
                nc.vector.tensor_copy(hdump[:], hT[:])
                nc.sync.dma_start(out=dbg["dbg_hT"], in_=hdump[:])
            # ---------------- Phase C: w_part = h @ W2_i + b2_i ---------
            # slab cols: [la2 | lb2 | lb1 | la1], within chunk col = r*128+dl
            ones_bf = S.tile([1, 128], BF16)
            nc.vector.memset(ones_bf[:], 1.0)
            wp = S.tile([B, 4096], FP32)
            for half in range(2):
                pws = []
                for c in range(4):
                    pwc = PW.tile([B, 512], FP32, tag=f"pw{c}", name=f"pw_{half}_{c}")
                    pws.append(pwc)
                for a in range(8):
                    slab = W2P.tile([128, 2048], BF16)
                    nc.sync.dma_start(out=slab[:], in_=w2[half, ts(a, 128), :])
                    for c in range(4):
                        nc.tensor.matmul(
                            pws[c][:],
                            hT[:, ts(a, 128)],
                            slab[:, ts(c, 512)],
                            start=(a == 0),
                            stop=False,
                        )
                for c in range(4):
                    cg = half * 4 + c
                    nc.tensor.matmul(
                        pws[c][:], ones_bf[:], b2_sb[:, ts(cg, 512)],
                        start=False, stop=True,
                    )
                    nc.vector.tensor_copy(wp[:, ts(cg, 512)], pws[c][:])

            if DEBUG:
                nc.sync.dma_start(out=dbg["dbg_wp"], in_=wp[:])
            # ---------------- t2 = sum_d la2*x -> AllReduce -------------
            prod = S.tile([B, DP], FP32)
            t2p = SM.tile([B, R], FP32)
            for r in range(R):
                nc.vector.scalar_tensor_tensor(
                    out=prod[:], in0=wp[:, ts(r, 128)], scalar=1.0,
                    in1=xcol_sb[:], op0=ALU.bypass, op1=ALU.mult,
                    accum_out=t2p[:, r : r + 1],
                )
            t2in = DR.tile([B, R], FP32)
            t2out = DR.tile([B, R], FP32)
            nc.sync.dma_start(out=t2in[:], in_=t2p[:])
            nc.gpsimd.collective_compute(
                "AllReduce", ALU.add, replica_groups=RG,
                ins=[t2in.opt()], outs=[t2out.opt()],
            )
            t2_sb = SM.tile([B, R], FP32)
            nc.sync.dma_start(out=t2_sb[:], in_=t2out[:])
            if DEBUG:
                nc.sync.dma_start(out=dbg["dbg_t2p"], in_=t2p[:])
                nc.sync.dma_start(out=dbg["dbg_t2"], in_=t2_sb[:])

            # ---------------- y = gelu(x@bd + sum_r lb2*t2) -------------
            py0 = PM.tile([B, 512], FP32, tag="ps")
            for dt_ in range(8):
                nc.tensor.matmul(
                    py0[:, 0:128],
                    xT_sb[:, ts(dt_, 128)],
                    bd_sb[:, ts(dt_, 128)],
                    start=(dt_ == 0),
                    stop=(dt_ == 7),
                )
            y_acc = SM.tile([B, DP], FP32)
            nc.vector.tensor_copy(y_acc[:], py0[:, 0:128])
            for r in range(R):
                nc.vector.scalar_tensor_tensor(
                    out=y_acc[:], in0=wp[:, 1024 + r * 128 : 1024 + (r + 1) * 128],
                    scalar=t2_sb[:, r : r + 1], in1=y_acc[:],
                    op0=ALU.mult, op1=ALU.add,
                )
            y_sb = S.tile([B, DP], FP32)
            nc.scalar.activation(out=y_sb[:], in_=y_acc[:], func=AF.Gelu)

            if DEBUG:
                nc.sync.dma_start(out=dbg["dbg_y"], in_=y_sb[:])
            # ---------------- t1 = sum_l lb1*y -> AllReduce -------------
            prod2 = S.tile([B, DP], FP32)
            t1p = SM.tile([B, R], FP32)
            for r in range(R):
                nc.vector.scalar_tensor_tensor(
                    out=prod2[:], in0=wp[:, 2048 + r * 128 : 2048 + (r + 1) * 128],
                    scalar=1.0, in1=y_sb[:], op0=ALU.bypass, op1=ALU.mult,
                    accum_out=t1p[:, r : r + 1],
                )
            t1in = DR.tile([B, R], FP32)
            t1out = DR.tile([B, R], FP32)
            nc.sync.dma_start(out=t1in[:], in_=t1p[:])
            nc.gpsimd.collective_compute(
                "AllReduce", ALU.add, replica_groups=RG,
                ins=[t1in.opt()], outs=[t1out.opt()],
            )
            t1_sb = SM.tile([B, R], FP32)
            nc.sync.dma_start(out=t1_sb[:], in_=t1out[:])

            if DEBUG:
                nc.sync.dma_start(out=dbg["dbg_t1"], in_=t1_sb[:])
            # ---------------- term1 = y_i @ buT -> ReduceScatter --------
            ident_f = S.tile([128, 128], FP32)
            make_identity(nc, ident_f[:])
            pyT = PM.tile([DP, 512], FP32, tag="ps")
            nc.tensor.transpose(pyT[:, 0:128], y_sb[:], ident_f[:])
            yT_sb = SM.tile([DP, B], FP32)
            nc.vector.tensor_copy(yT_sb[:], pyT[:, 0:128])
            rs_sb = S.tile([B, D], BF16)
            pterms = []
            for n in range(2):
                ptn = PM.tile([B, 512], FP32, tag="ps", name=f"pterm{n}")
                pterms.append(ptn)
            for n in range(2):
                nc.tensor.matmul(
                    pterms[n][:], yT_sb[:], buT_sb[:, ts(n, 512)],
                    start=True, stop=True,
                )
                nc.vector.tensor_copy(rs_sb[:, ts(n, 512)], pterms[n][:])
            rsin = DR.tile([B, D], BF16)
            rsout = DR.tile([BP, D], BF16)
            nc.sync.dma_start(out=rsin[:], in_=rs_sb[:])
            nc.gpsimd.collective_compute(
                "ReduceScatter", ALU.add, replica_groups=RG,
                ins=[rsin.opt()], outs=[rsout.opt()],
            )
            if DEBUG:
                nc.sync.dma_start(out=dbg["dbg_rs"], in_=rs_sb[:])
            # ---------------- term2 = sum_r la1*t1, scattered into rs ----
            term2 = SM.tile([B, DP], FP32)
            for r in range(R):
                sl = wp[:, 3072 + r * 128 : 3072 + (r + 1) * 128]
                if r == 0:
                    nc.vector.tensor_scalar_mul(
                        out=term2[:], in0=sl, scalar1=t1_sb[:, r : r + 1]
                    )
                else:
                    nc.vector.scalar_tensor_tensor(
                        out=term2[:], in0=sl, scalar=t1_sb[:, r : r + 1],
                        in1=term2[:], op0=ALU.mult, op1=ALU.add,
                    )
            if DEBUG:
                nc.sync.dma_start(out=dbg["dbg_term2"], in_=term2[:])
            a2in = DR.tile([B, DP], FP32)
            a2out = DR.tile([B, DP], FP32)
            nc.sync.dma_start(out=a2in[:], in_=term2[:])
            nc.gpsimd.collective_compute(
                "AllToAll", ALU.bypass, replica_groups=RG,
                ins=[a2in.opt()], outs=[a2out.opt()],
            )

            # ---------------- final: out = x + rsout + term2(asm) -------
            fin_bf = SM.tile([BP, D], BF16)
            nc.sync.dma_start(out=fin_bf[:], in_=rsout[:])
            fin = SM.tile([BP, D], FP32)
            # a2out rows = [src_rank j][16 samples]; cols = rank j's k-slice
            nc.sync.dma_start(
                out=fin[:].rearrange("b (j k) -> b j k", j=8),
                in_=a2out[:].rearrange("(j b) k -> b j k", j=8),
            )
            nc.vector.tensor_add(fin[:], fin[:], xres_sb[:])
            fin2 = SM.tile([BP, D], FP32)
            nc.vector.tensor_add(fin2[:], fin[:], fin_bf[:])
            nc.sync.dma_start(out=outp, in_=fin2[:])

    _split_excess_waits(nc)
    return nc


_NC = None


def _get_nc():
    global _NC
    if _NC is None:
        _NC = build_graph()
    return _NC


def make_in_maps(x, ada_emb, base_up, base_down, ln_g, ln_b, W1, b1, W2, b2):
    f32 = np.float32
    bf16 = ml_dtypes.bfloat16
    x = np.asarray(x, f32)
    ada_emb = np.ascontiguousarray(np.asarray(ada_emb, f32))
    W1 = np.asarray(W1, f32)
    W2 = np.asarray(W2, f32)
    # fold LN affine into W1/b1
    W1p = (np.asarray(ln_g, f32)[:, None] * W1).astype(bf16)
    b1p = (np.asarray(b1, f32) + np.asarray(ln_b, f32) @ W1).astype(f32)
    b1c = np.ascontiguousarray(b1p.reshape(8, 128).T)
    xT = np.ascontiguousarray(x.T)
    W2r = W2.reshape(ADA, 4, D, R)
    b2r_full = np.asarray(b2, f32).reshape(4, D, R)
    chunk_order = [2, 3, 1, 0]  # la2, lb2, lb1, la1

    in_maps = []
    for i in range(NCORES):
        dl = slice(i * DP, (i + 1) * DP)
        sl = W2r[:, chunk_order, dl, :]            # [a, 4, 128, 8]
        sl = sl.transpose(1, 3, 2, 0)              # [4, r, dl, a]
        # cols (chunk, r, dl): slab[a, chunk*1024 + r*128 + dl]
        slab = np.ascontiguousarray(sl.reshape(4096, ADA).T)   # [a, 4096]
        w2i = np.ascontiguousarray(
            slab.reshape(ADA, 2, 2048).transpose(1, 0, 2)
        ).astype(bf16)                              # [2, a, 2048]
        b2sl = b2r_full[chunk_order, dl, :]         # [4, 128, 8]
        b2i = np.ascontiguousarray(
            b2sl.transpose(0, 2, 1).reshape(1, 4096)
        ).astype(bf16)
        in_maps.append(
            {
                "ada": ada_emb,
                "w1": W1p,
                "b1c": b1c,
                "w2": w2i,
                "b2r": b2i,
                "xT": xT,
                "xcol": np.ascontiguousarray(x[:, dl]),
                "bd": np.ascontiguousarray(np.asarray(base_down, f32)[:, dl]),
                "buT": np.ascontiguousarray(np.asarray(base_up, f32)[:, dl].T),
                "xres": np.ascontiguousarray(x[i * BP : (i + 1) * BP, :]),
            }
        )
    return in_maps


def run(trace=False, tmpdir=None, **inputs):
    nc = _get_nc()
    in_maps = make_in_maps(**inputs)
    res = bass_utils.run_bass_kernel_spmd(
        nc, in_maps, core_ids=list(range(NCORES)), trace=trace, tmpdir=tmpdir
    )
    out = np.concatenate([res.results[i]["out"] for i in range(NCORES)], axis=0)
    return out, res


def kernel(**inputs):
    out, _ = run(trace=False, **inputs)
    return out
